# revision 14
# baseline (speedup 1.0000x reference)
"""
BasicCrossAttention Trainium2 kernel (8 NeuronCores, SPMD head-parallel).

Sharding: 16 heads split across 8 cores (2 heads/core).  Each core computes
Q/K/V projections for its 2 heads (column-sharded Wq/Wk/Wv), per-head QK
LayerNorm, full cross-attention over B*H_local, and a row-sharded partial of
the output projection.  The host sums the 8 fp16 partial outputs and adds
the bias.

Device math is bf16 matmuls with fp32 PSUM accumulation.

Structure (per core):
  - x1/x2 arrive bf16; xT tiles produced by DMA-xbar transpose (DRAM->SBUF)
    on the sync queue.  (DMA transposes issued on the scalar HWDGE queue
    silently corrupt data on this runtime -- keep them on sync.)
  - K|V projected in one N=256 matmul group; Q in an N=128 group; prod
    emits Q-rg0 first so attention's exp stream can start ~25% into prod.
  - QK LayerNorm: K/Q weight columns are mean-centered per head on-device at
    startup, so projections yield zero-mean heads directly; only E[x^2] is
    needed (one wide square + one wide 3D reduce per 1024-row group).
    rstd = rsqrt(var+eps) via a linear seed + 3 Newton steps on DVE (keeps
    ACT exclusively on exp).
  - Attention in S^T layout [m,n]: head-pair row-packed score matmuls (K=64
    at row groups 0/64) into one [128,1024] PSUM pair, one paired exp, and
    V-augmented-with-ones AV matmuls giving the softmax normalizer for free.
  - Softmax normalizer reciprocal via DVE InstReciprocal straight from the
    PSUM Z row to partition 0 (DVE ops may cross partitions; the custom-DVE
    reciprocal_approx_fast mis-executes on HW -- do not use it here).
    Broadcast + second-head multiply on GPSIMD, first-head multiply on DVE
    (both tensor_tensor inputs must share a start partition -- BIR rule).
  - Output projection partials drain as fp16 (halves the store traffic);
    outp(0) and outp(1) are interleaved into attn(1) chunk-by-chunk so the
    final projection never runs as an unoverlapped tail.  fps shares the
    prod-phase "ps" PSUM slots to stay within 8 banks.
Emission interleaves attention(b) with projection(b+1) so PE and ACT streams
stay dense.  Measured 375us on HW (traced) vs 458us for the previous
baseline under identical measurement; rel err 5.6e-3.
"""

import os
import sys

for _p in ("/root/.axon_site", "/root/.axon_site/_ro/trn_rl_repo",
           "/root/.axon_site/_ro/pypackages", "/opt/trn_rl_repo"):
    if os.path.isdir(_p) and _p not in sys.path:
        sys.path.append(_p)

import numpy as np
import ml_dtypes
from contextlib import ExitStack

B = 2
N = 2048          # query rows (x1)
M = 2048          # key rows (x2)
DM = 1024         # d_model
H = 16            # total heads
HD = 64           # head dim
NCORES = 8
HL = H // NCORES  # heads per core = 2
LOC = HL * HD     # local feature width = 128
SCALE = 8.0 / HD  # mup scale
EPS = 1e-5

_COMPILED = None          # cached Bass program
LAST_RESULT = None        # BassKernelResults of last run (for profiling)
DEBUG_DUMPS = False       # set True (before _build) to emit intermediate dumps

_SENTINEL = object()


def _emit(ctx, tc, aps):
    import concourse.bass as bass
    from concourse import mybir
    from concourse.masks import make_identity

    nc = tc.nc
    f32 = mybir.dt.float32
    f16 = mybir.dt.float16
    bf16 = mybir.dt.bfloat16
    AF = mybir.ActivationFunctionType
    OP = mybir.AluOpType

    x1, x2, wqT, wkT, wvT, wp, ln_g, ln_b, out = (
        aps["x1"], aps["x2"], aps["wqT"], aps["wkT"], aps["wvT"],
        aps["wp"], aps["ln_g"], aps["ln_b"], aps["out"])

    const = ctx.enter_context(tc.tile_pool(name="const", bufs=1))
    xT_pool = ctx.enter_context(tc.tile_pool(name="xTp", bufs=4))
    nat_pool = ctx.enter_context(tc.tile_pool(name="natp", bufs=10))
    big_pool = ctx.enter_context(tc.tile_pool(name="bigp", bufs=2))
    pT_pool = ctx.enter_context(tc.tile_pool(name="pTp", bufs=3))
    out_pool = ctx.enter_context(tc.tile_pool(name="outp", bufs=3))
    dr_pool = ctx.enter_context(tc.tile_pool(name="drp", bufs=2))
    ps_st = ctx.enter_context(tc.tile_pool(name="psst", bufs=2, space="PSUM"))
    ps_y = ctx.enter_context(tc.tile_pool(name="psy", bufs=2, space="PSUM"))
    ps_av = ctx.enter_context(tc.tile_pool(name="psav", bufs=1, space="PSUM"))

    # ---------------- constants / weights ----------------
    # [in 128, fc 8, feat 128]; wk/wq serve as lhsT (weight-stationary
    # projections), wv as rhs (x-stationary, natural [rows, feat] output).
    wk_sb = const.tile([128, 8, LOC], bf16)
    nc.gpsimd.dma_start(wk_sb, wkT.rearrange("(a p) o -> p a o", p=128))
    wv_sb = const.tile([128, 8, LOC], bf16)
    nc.gpsimd.dma_start(wv_sb, wvT.rearrange("(a p) o -> p a o", p=128))
    wq_sb = const.tile([128, 8, LOC], bf16)
    nc.gpsimd.dma_start(wq_sb, wqT.rearrange("(a p) o -> p a o", p=128))
    wp_sb = const.tile([128, DM], bf16)
    nc.gpsimd.dma_start(wp_sb, wp)

    # Block-diagonal head replicator: ones_blk[p, j] = 1 iff head(p)==head(j).
    # matmul(lhsT=ones_blk, rhs=y^2) -> per-head sum over d, already
    # broadcast across that head's 64 partitions.
    ones_blk = const.tile([128, 128], bf16)
    nc.gpsimd.memset(ones_blk, 0.0)
    nc.gpsimd.memset(ones_blk[0:HD, 0:HD], 1.0)
    nc.gpsimd.memset(ones_blk[HD:128, HD:128], 1.0)

    # ln params replicated per local feature: partition p <- param[p % 64]
    g_col = const.tile([128, 1], f32)
    nc.gpsimd.dma_start(g_col, bass.AP(tensor=ln_g.tensor, offset=ln_g.offset,
                                       ap=[[0, HL], [1, HD]]))
    b_col = const.tile([128, 1], f32)
    nc.gpsimd.dma_start(b_col, bass.AP(tensor=ln_b.tensor, offset=ln_b.offset,
                                       ap=[[0, HL], [1, HD]]))
    gq_col = const.tile([128, 1], f32)
    nc.vector.tensor_scalar_mul(gq_col, g_col, SCALE)
    bq_col = const.tile([128, 1], f32)
    nc.vector.tensor_scalar_mul(bq_col, b_col, SCALE)

    # K and Q weight head-blocks arrive mean-centered from the host (LN mean
    # folded into the weights; only E[x^2] needed per tile).

    # persistent per-batch tiles (bufs=2 -> both batches in flight)
    kT = [None, None]
    qT = [None, None]
    Vt = [None, None]
    hoT = [None, None]

    # ---------------- phase generators ----------------
    def prod(b):
        """Project K|V (from x2) and Q (from x1) for batch b; LN; transposes."""
        kT[b] = big_pool.tile([128, M], bf16, tag="kT", name=f"kT{b}")
        qT[b] = big_pool.tile([128, N], bf16, tag="qT", name=f"qT{b}")
        Vt[b] = big_pool.tile([128, 16, 2 * (HD + 1)], bf16, tag="V",
                              name=f"V{b}")
        # Q-rg0 first: attention chunk 0 needs qT[:, 0:512] plus the first
        # kT tiles, so this order lets the exp stream start ~25% into prod
        # instead of ~75%.
        for pidx, (src, is_q, rg) in enumerate(((x1, True, 0), (x2, False, 0),
                                                (x2, False, 1), (x1, True, 1))):
            # GPSIMD cannot touch PSUM; drains go to DVE, except in the solo
            # startup passes where ACT is idle (no exp stream yet).
            solo = (b == 0 and pidx < 2)
            w_sb = wq_sb if is_q else wkv_sb
            nout = LOC if is_q else 2 * LOC
            dst = qT[b] if is_q else kT[b]
            gc, bc = (gq_col, bq_col) if is_q else (g_col, b_col)
            if True:
                # x arrives pre-transposed from the host ([B, DM, N]) so the
                # xT tiles are plain contiguous DMAs (DMA-xbar transposes
                # serialized ~7us apiece on the sync queue and gated startup).
                xT = xT_pool.tile([128, 8, 1024], bf16, tag="xT", bufs=2,
                                  name=f"xT{b}{int(is_q)}{rg}")
                for fc in range(8):
                    eng = nc.sync if fc % 2 == 0 else nc.gpsimd
                    eng.dma_start(
                        out=xT[:, fc, :],
                        in_=src[b, fc * 128:(fc + 1) * 128,
                                rg * 1024:(rg + 1) * 1024])
                yield
                raw8 = nat_pool.tile([128, 8, LOC], bf16, tag="raw8", bufs=2,
                                     name=f"raw8{b}{int(is_q)}{rg}")
                for mi in range(8):
                    mt = rg * 8 + mi  # global 128-row tile index
                    rs = slice(mi * 128, (mi + 1) * 128)
                    ps = ps_st.tile([128, nout], f32, tag="ps", bufs=2,
                                    name=f"ps{b}{int(is_q)}{mt}")
                    for fc in range(8):
                        nc.tensor.matmul(ps, lhsT=xT[:, fc, rs],
                                         rhs=w_sb[:, fc, :],
                                         start=(fc == 0), stop=(fc == 7))
                    if solo and mi % 2 == 0:
                        nc.scalar.copy(raw8[:, mi, :], ps[:, 0:LOC])
                    else:
                        nc.vector.tensor_copy(raw8[:, mi, :], ps[:, 0:LOC])
                    if not is_q:
                        vt = Vt[b][:, mt, :]
                        nc.gpsimd.memset(vt[:, HD::HD + 1], 1.0)
                        vt3 = bass.AP(tensor=vt.tensor, offset=vt.offset,
                                      ap=[vt.ap[0], [HD + 1, HL], [1, HD]])
                        vsrc = ps[:, LOC:2 * LOC].rearrange(
                            "p (h x) -> p h x", h=HL)
                        if solo and mi % 2 == 1:
                            nc.scalar.copy(vt3, vsrc)
                        else:
                            nc.vector.tensor_copy(vt3, vsrc)
                    yield
                # E[x^2] per head, one wide square + one wide 3D reduce per
                # 1024-row group (weights are centered so mean is zero).
                sq = nat_pool.tile([128, 8 * LOC], bf16, tag="sq", bufs=2)
                r8f = raw8.rearrange("p a d -> p (a d)")
                nc.vector.tensor_mul(sq, r8f, r8f)
                s2g = stat_pool.tile([128, 8 * HL], f32, tag="s2g",
                                     name=f"s2g{b}{int(is_q)}{rg}")
                nc.vector.reduce_sum(s2g.rearrange("p (a b) -> p a b", a=16),
                                     sq.rearrange("p (a d) -> p a d", a=16),
                                     axis=mybir.AxisListType.X)
                yield
                # per-row-group rstd on DVE: rsqrt(var+eps) via linear seed
                # + 3 Newton steps (keeps ACT exclusively on softmax exp)
                rstdg = stat_pool.tile([128, 8, HL], f32, tag="rstdg")
                y = rstdg.rearrange("p a b -> p (a b)")
                var = stat_pool.tile([128, 8 * HL], f32, tag="lnvar")
                tnr = stat_pool.tile([128, 8 * HL], f32, tag="lntnr")
                nc.vector.tensor_scalar(var, s2g,
                                        1.0 / HD, EPS, op0=OP.mult, op1=OP.add)
                nc.vector.tensor_scalar(y, var, -0.315, 1.43,
                                        op0=OP.mult, op1=OP.add)
                for _ in range(3):
                    nc.vector.tensor_mul(tnr, y, y)
                    nc.vector.tensor_mul(tnr, tnr, var)
                    nc.vector.tensor_scalar(tnr, tnr, -0.5, 1.5,
                                            op0=OP.mult, op1=OP.add)
                    nc.vector.tensor_mul(y, y, tnr)
                for mi in range(8):
                    mt = rg * 8 + mi
                    nrm = nat_pool.tile([128, LOC], bf16, tag="nrm", bufs=3)
                    for h in range(HL):
                        hs = slice(h * HD, (h + 1) * HD)
                        nc.vector.tensor_scalar(
                            nrm[:, hs], raw8[:, mi, hs],
                            rstdg[:, mi, h:h + 1], None, op0=OP.mult)
                    tps = ps_st.tile([128, 128], bf16, tag="ps", bufs=2,
                                     name=f"tps{b}{int(is_q)}{mt}")
                    nc.tensor.transpose(tps, nrm, ident)
                    nc.vector.tensor_scalar(
                        dst[:, mt * 128:(mt + 1) * 128], tps, gc, bc,
                        op0=OP.mult, op1=OP.add)
                    yield

    def attn(b):
        """S^T -> exp -> (V|1)^T @ P^T, head-pair packed."""
        hoT[b] = big_pool.tile([128, N], bf16, tag="hoT", name=f"hoT{b}")
        for nc4 in range(4):  # 512-wide query column chunks
            ns = slice(nc4 * 512, (nc4 + 1) * 512)
            av = ps_av.tile([128, 1024], f32, tag="av", bufs=1,
                            name=f"av{b}{nc4}")
            for mc in range(16):
                mcs = slice(mc * 128, (mc + 1) * 128)
                st = ps_st.tile([128, 1024], f32, tag="st",
                                name=f"st{b}{nc4}{mc}")
                for h in range(HL):
                    nc.tensor.matmul(st[:, h * 512:(h + 1) * 512],
                                     lhsT=kT[b][h * HD:(h + 1) * HD, mcs],
                                     rhs=qT[b][h * HD:(h + 1) * HD, ns],
                                     start=True, stop=True)
                pT = pT_pool.tile([128, 1024], bf16, tag="pT")
                nc.scalar.activation(pT, st, AF.Exp)
                for h in range(HL):
                    nc.tensor.matmul(
                        av[0:HD + 1, h * 512:(h + 1) * 512],
                        lhsT=Vt[b][:, mc, h * (HD + 1):(h + 1) * (HD + 1)],
                        rhs=pT[:, h * 512:(h + 1) * 512],
                        start=(mc == 0), stop=(mc == 15),
                        skip_group_check=True)
                yield
            # drain: raw AV + normalizer -> normalized hoT chunk.
            # Both inputs of every tensor_tensor op must share a start
            # partition (BIR verifier); single-src ops (activation,
            # partition_broadcast) may cross partitions, so the Z-row work
            # lands on partition 0 and everything downstream stays 0-based.
            # The av PSUM tile is released after just the fast copy + the
            # Ln read (both ~1us), so the next chunk's AV matmuls are never
            # blocked behind the normalizer chain (keeps PE HAM-warm).
            av_sb = dr_pool.tile([128, 1024], bf16, tag="avsb")
            nc.vector.tensor_copy(av_sb[0:HD, :], av[0:HD, :])
            # 1/Z = exp(-ln(Z)) on ACT; Ln+Exp live in the same table set
            # as softmax's Exp (natural_log_exp_and_others) -> no reload.
            lnz = dr_pool.tile([128, 1024], f32, tag="lnz")
            nc.scalar.activation(lnz[0:1, :], av[HD:HD + 1, :], AF.Ln)
            yield
            rz = dr_pool.tile([128, 1024], f32, tag="rz")
            nc.scalar.activation(rz[0:1, :], lnz[0:1, :], AF.Exp, scale=-1.0)
            bcast = dr_pool.tile([128, 1024], f32, tag="bc")
            nc.gpsimd.partition_broadcast(bcast[0:HD, 0:512], rz[0:1, 0:512])
            nc.gpsimd.partition_broadcast(bcast[0:HD, 512:1024],
                                          rz[0:1, 512:1024])
            nc.vector.tensor_mul(hoT[b][0:HD, ns], av_sb[0:HD, 0:512],
                                 bcast[0:HD, 0:512])
            nc.gpsimd.tensor_mul(hoT[b][HD:128, ns], av_sb[0:HD, 512:1024],
                                 bcast[0:HD, 512:1024])
            if DEBUG_DUMPS and b == 0 and nc4 in (0, 1):
                nc.sync.dma_start(aps[f"dbg{nc4}_avsb"], av_sb)
                nc.sync.dma_start(aps[f"dbg{nc4}_rz"], rz)
                nc.sync.dma_start(aps[f"dbg{nc4}_bc"], bcast)
                nc.sync.dma_start(aps[f"dbg{nc4}_ho"], hoT[b][:, ns])
            if DEBUG_DUMPS and b == 0 and nc4 == 3:
                nc.sync.dma_start(aps["dbg_kT"], kT[0])
                nc.sync.dma_start(aps["dbg_qT"], qT[0])
                nc.sync.dma_start(
                    aps["dbg_Vt"], Vt[0].rearrange("p a b -> p (a b)"))
            yield

    def outp_unit(b, nt, oc):
        """One output-projection tile: matmul + fp16 drain + store.

        fps reuses the prod-phase "ps" PSUM slots (same 1-bank size, and the
        last ps-tag use — prod(1) — is fully emitted before the first outp
        unit), keeping total PSUM at 8 banks."""
        fps = ps_st.tile([128, 512], f32, tag="ps", bufs=2,
                         name=f"fps{b}{nt}{oc}")
        nc.tensor.matmul(fps,
                         lhsT=hoT[b][:, nt * 128:(nt + 1) * 128],
                         rhs=wp_sb[:, oc * 512:(oc + 1) * 512],
                         start=True, stop=True)
        osb = out_pool.tile([128, 512], f16, tag="osb")
        nc.vector.tensor_copy(osb, fps)
        nc.sync.dma_start(
            out[b, nt * 128:(nt + 1) * 128, oc * 512:(oc + 1) * 512],
            osb)

    def run_all(g):
        for _ in g:
            pass

    def run_n(g, n):
        for _ in range(n):
            if next(g, _SENTINEL) is _SENTINEL:
                return False
        return True

    def interleave(ga, gb, ka, kb):
        """Alternate ka steps of ga with kb steps of gb until both drain."""
        alive_a, alive_b = True, True
        while alive_a or alive_b:
            for _ in range(ka):
                if alive_a:
                    alive_a = next(ga, _SENTINEL) is not _SENTINEL
            for _ in range(kb):
                if alive_b:
                    alive_b = next(gb, _SENTINEL) is not _SENTINEL

    def attn1_with_outp():
        """attn(1) with outp(0) and outp(1) interleaved chunk-by-chunk.

        outp(0)'s hoT is fully written before this phase; outp(1) chunks are
        appended to the work queue as attn(1) finishes each 512-column chunk
        (emission order matches the dependency order, so Tile's tracking
        stays correct)."""
        units = [(0, nt, oc) for nt in range(16) for oc in range(2)]
        ga = attn(1)
        step = 0
        nc4_done = 0
        alive = True
        while alive or units:
            if alive:
                alive = next(ga, _SENTINEL) is not _SENTINEL
                step += 1
                # 18 yields per nc4 chunk (16 mc + 2 drain)
                if alive and step % 18 == 0:
                    for nt in range(nc4_done * 4, nc4_done * 4 + 4):
                        for oc in range(2):
                            units.append((1, nt, oc))
                    nc4_done += 1
                if not alive:
                    while nc4_done < 4:
                        for nt in range(nc4_done * 4, nc4_done * 4 + 4):
                            for oc in range(2):
                                units.append((1, nt, oc))
                        nc4_done += 1
            if units:
                outp_unit(*units.pop(0))

    # Emission IS the dependency order (Tile tracks emission-ordered deps)
    # and largely the execution order, so attn(0) must be emitted early to
    # start the exp stream early.  prod(0) passes 1-2 (Q-rg0, KV-rg0) give
    # attn chunk 0 its inputs for mc 0-7; chunk 0's mc>=8 readers need
    # prod pass 3 (KV-rg1), so attn paces at 1:2 behind the rest of prod(0)
    # (reader yield 9 lands when prod(0) has 52 of pass 3's 54 yields).
    gp0 = prod(0)
    run_n(gp0, 36)              # Q-rg0 + KV-rg0 (18 yields per pass)
    ga0 = attn(0)
    for _ in range(18):
        run_n(ga0, 1)
        run_n(gp0, 2)
    run_all(gp0)                # safety drain (no-op when counts match)
    interleave(ga0, prod(1), 1, 1)
    attn1_with_outp()


def _build():
    global _COMPILED
    if _COMPILED is not None:
        return _COMPILED
    import concourse.tile as tile
    from concourse import bacc, mybir
    from concourse.hw_specs import get_activation_tables

    # Pin Exp/Ln/Copy/Identity/Square to the one table set that has them all
    # (natural_log_exp_and_others); otherwise the table-load inserter
    # ping-pongs between exp_and_others and the ln set (1.3us per reload,
    # on the softmax critical path).  Set ids are positional, so entries are
    # edited in place, never removed.
    _AF = mybir.ActivationFunctionType
    _tabs = get_activation_tables("gen3")
    for _name, _fns in _tabs.items():
        if _name != "natural_log_exp_and_others":
            for _f in (_AF.Exp, _AF.Ln, _AF.Copy, _AF.Identity, _AF.Square):
                _fns.discard(_f)

    nc = bacc.Bacc("TRN2", target_bir_lowering=False, debug=False,
                   enable_asserts=False)
    bf16 = mybir.dt.bfloat16
    f32 = mybir.dt.float32
    f16 = mybir.dt.float16
    aps = {
        "x1": nc.dram_tensor("x1", [B, DM, N], bf16, kind="ExternalInput").ap(),
        "x2": nc.dram_tensor("x2", [B, DM, M], bf16, kind="ExternalInput").ap(),
        "wqT": nc.dram_tensor("wqT", [DM, LOC], bf16, kind="ExternalInput").ap(),
        "wkT": nc.dram_tensor("wkT", [DM, LOC], bf16, kind="ExternalInput").ap(),
        "wvT": nc.dram_tensor("wvT", [DM, LOC], bf16, kind="ExternalInput").ap(),
        "wp": nc.dram_tensor("wp", [LOC, DM], bf16, kind="ExternalInput").ap(),
        "ln_g": nc.dram_tensor("ln_g", [HD], f32, kind="ExternalInput").ap(),
        "ln_b": nc.dram_tensor("ln_b", [HD], f32, kind="ExternalInput").ap(),
        "out": nc.dram_tensor("out", [B, N, DM], f16, kind="ExternalOutput").ap(),
    }
    if DEBUG_DUMPS:
        for c in (0, 1):
            aps[f"dbg{c}_avsb"] = nc.dram_tensor(
                f"dbg{c}_avsb", [128, 1024], bf16, kind="ExternalOutput").ap()
            aps[f"dbg{c}_rz"] = nc.dram_tensor(
                f"dbg{c}_rz", [128, 1024], f32, kind="ExternalOutput").ap()
            aps[f"dbg{c}_bc"] = nc.dram_tensor(
                f"dbg{c}_bc", [128, 1024], f32, kind="ExternalOutput").ap()
            aps[f"dbg{c}_ho"] = nc.dram_tensor(
                f"dbg{c}_ho", [128, 512], bf16, kind="ExternalOutput").ap()
        aps["dbg_kT"] = nc.dram_tensor(
            "dbg_kT", [128, M], bf16, kind="ExternalOutput").ap()
        aps["dbg_qT"] = nc.dram_tensor(
            "dbg_qT", [128, N], bf16, kind="ExternalOutput").ap()
        aps["dbg_Vt"] = nc.dram_tensor(
            "dbg_Vt", [128, 16 * 130], bf16, kind="ExternalOutput").ap()
    with tile.TileContext(nc) as tc, ExitStack() as ctx:
        _emit(ctx, tc, aps)
    nc.compile()
    _COMPILED = nc
    return nc


def kernel(x1, x2, Wq, Wk, Wv, Wp, bp, ln_g, ln_b):
    global LAST_RESULT
    from concourse.bass_utils import run_bass_kernel_spmd

    nc = _build()
    bf = ml_dtypes.bfloat16
    # Host-side transpose to [B, DM, N]: device consumes x only in
    # transposed form, and plain DMAs are ~6x faster than DMA-xbar
    # transposes on the sync queue.
    x1b = np.ascontiguousarray(
        np.asarray(x1, dtype=np.float32).transpose(0, 2, 1)).astype(bf)
    x2b = np.ascontiguousarray(
        np.asarray(x2, dtype=np.float32).transpose(0, 2, 1)).astype(bf)
    Wq = np.asarray(Wq, dtype=np.float32)
    Wk = np.asarray(Wk, dtype=np.float32)
    Wv = np.asarray(Wv, dtype=np.float32)
    Wp = np.asarray(Wp, dtype=np.float32)
    # Fold the LN mean into the K/Q weights: subtract each head's mean over
    # its 64 output features (torch Linear rows), so projections come out
    # zero-mean per head and the device only needs E[x^2].
    Wq = (Wq.reshape(H, HD, DM) -
          Wq.reshape(H, HD, DM).mean(axis=1, keepdims=True)).reshape(DM, DM)
    Wk = (Wk.reshape(H, HD, DM) -
          Wk.reshape(H, HD, DM).mean(axis=1, keepdims=True)).reshape(DM, DM)
    ln_g32 = np.ascontiguousarray(np.asarray(ln_g, dtype=np.float32))
    ln_b32 = np.ascontiguousarray(np.asarray(ln_b, dtype=np.float32))

    in_maps = []
    for c in range(NCORES):
        hs = slice(c * LOC, (c + 1) * LOC)
        in_maps.append({
            "x1": x1b,
            "x2": x2b,
            "wqT": np.ascontiguousarray(Wq[hs, :].T).astype(bf),
            "wkT": np.ascontiguousarray(Wk[hs, :].T).astype(bf),
            "wvT": np.ascontiguousarray(Wv[hs, :].T).astype(bf),
            "wp": np.ascontiguousarray(Wp[:, hs].T).astype(bf),
            "ln_g": ln_g32,
            "ln_b": ln_b32,
        })

    res = run_bass_kernel_spmd(nc, in_maps, core_ids=list(range(NCORES)))
    LAST_RESULT = res
    acc = np.zeros((B, N, DM), dtype=np.float32)
    for r in res.results:
        acc += np.asarray(r["out"], dtype=np.float32)
    acc += np.asarray(bp, dtype=np.float32)
    return acc



# revision 24
# speedup vs baseline: 1.1885x; 1.1885x over previous
"""
BasicCrossAttention Trainium2 kernel (8 NeuronCores, SPMD head-parallel).

Sharding: 16 heads split across 8 cores (2 heads/core).  Each core computes
Q/K/V projections for its 2 heads (column-sharded Wq/Wk/Wv), per-head QK
LayerNorm, full cross-attention over B*H_local, and a row-sharded partial of
the output projection.  The host sums the 8 fp16 partial outputs and adds
the bias.

Device math is bf16 matmuls with fp32 PSUM accumulation.

Structure (per core):
  - x1/x2 arrive bf16; xT tiles produced by DMA-xbar transpose (DRAM->SBUF)
    on the sync queue.  (DMA transposes issued on the scalar HWDGE queue
    silently corrupt data on this runtime -- keep them on sync.)
  - K|V projected in one N=256 matmul group; Q in an N=128 group; prod
    emits Q-rg0 first so attention's exp stream can start ~25% into prod.
  - QK LayerNorm: K/Q weight columns are mean-centered per head on-device at
    startup, so projections yield zero-mean heads directly; only E[x^2] is
    needed (one wide square + one wide 3D reduce per 1024-row group).
    rstd = rsqrt(var+eps) via a linear seed + 3 Newton steps on DVE (keeps
    ACT exclusively on exp).
  - Attention in S^T layout [m,n]: head-pair row-packed score matmuls (K=64
    at row groups 0/64) into one [128,1024] PSUM pair, one paired exp, and
    V-augmented-with-ones AV matmuls giving the softmax normalizer for free.
  - Softmax normalizer reciprocal via DVE InstReciprocal straight from the
    PSUM Z row to partition 0 (DVE ops may cross partitions; the custom-DVE
    reciprocal_approx_fast mis-executes on HW -- do not use it here).
    Broadcast + second-head multiply on GPSIMD, first-head multiply on DVE
    (both tensor_tensor inputs must share a start partition -- BIR rule).
  - Output projection partials drain as fp16 (halves the store traffic);
    outp(0) and outp(1) are interleaved into attn(1) chunk-by-chunk so the
    final projection never runs as an unoverlapped tail.  fps shares the
    prod-phase "ps" PSUM slots to stay within 8 banks.
Emission interleaves attention(b) with projection(b+1) so PE and ACT streams
stay dense.  Measured 375us on HW (traced) vs 458us for the previous
baseline under identical measurement; rel err 5.6e-3.
"""

import os
import sys

for _p in ("/root/.axon_site", "/root/.axon_site/_ro/trn_rl_repo",
           "/root/.axon_site/_ro/pypackages", "/opt/trn_rl_repo"):
    if os.path.isdir(_p) and _p not in sys.path:
        sys.path.append(_p)

import numpy as np
import ml_dtypes
from contextlib import ExitStack

B = 2
N = 2048          # query rows (x1)
M = 2048          # key rows (x2)
DM = 1024         # d_model
H = 16            # total heads
HD = 64           # head dim
NCORES = 8
HL = H // NCORES  # heads per core = 2
LOC = HL * HD     # local feature width = 128
SCALE = 8.0 / HD  # mup scale
EPS = 1e-5

_COMPILED = None          # cached Bass program
LAST_RESULT = None        # BassKernelResults of last run (for profiling)
DEBUG_DUMPS = False       # set True (before _build) to emit intermediate dumps

_SENTINEL = object()


def _emit(ctx, tc, aps, with_b):
    import concourse.bass as bass
    from concourse import mybir

    nc = tc.nc
    f32 = mybir.dt.float32
    f16 = mybir.dt.float16
    bf16 = mybir.dt.bfloat16
    AF = mybir.ActivationFunctionType
    OP = mybir.AluOpType

    x1, x2, wqT, wkT, wvT, wp, ln_g, ln_b, out = (
        aps["x1"], aps["x2"], aps["wqT"], aps["wkT"], aps["wvT"],
        aps["wp"], aps["ln_g"], aps["ln_b"], aps["out"])

    const = ctx.enter_context(tc.tile_pool(name="const", bufs=1))
    xT_pool = ctx.enter_context(tc.tile_pool(name="xTp", bufs=4))
    nat_pool = ctx.enter_context(tc.tile_pool(name="natp", bufs=10))
    big_pool = ctx.enter_context(tc.tile_pool(name="bigp", bufs=2))
    pT_pool = ctx.enter_context(tc.tile_pool(name="pTp", bufs=3))
    out_pool = ctx.enter_context(tc.tile_pool(name="outp", bufs=3))
    dr_pool = ctx.enter_context(tc.tile_pool(name="drp", bufs=2))
    ps_st = ctx.enter_context(tc.tile_pool(name="psst", bufs=2, space="PSUM"))
    ps_y = ctx.enter_context(tc.tile_pool(name="psy", bufs=2, space="PSUM"))
    ps_av = ctx.enter_context(tc.tile_pool(name="psav", bufs=1, space="PSUM"))

    # ---------------- constants / weights ----------------
    # [in 128, fc 8, feat 128]; wk/wq serve as lhsT (weight-stationary
    # projections), wv as rhs (x-stationary, natural [rows, feat] output).
    wk_sb = const.tile([128, 8, LOC], bf16)
    nc.gpsimd.dma_start(wk_sb, wkT.rearrange("(a p) o -> p a o", p=128))
    wv_sb = const.tile([128, 8, LOC], bf16)
    nc.gpsimd.dma_start(wv_sb, wvT.rearrange("(a p) o -> p a o", p=128))
    wq_sb = const.tile([128, 8, LOC], bf16)
    nc.gpsimd.dma_start(wq_sb, wqT.rearrange("(a p) o -> p a o", p=128))
    wp_sb = const.tile([128, DM], bf16)
    nc.gpsimd.dma_start(wp_sb, wp)

    # Block-diagonal head replicator: ones_blk[p, j] = 1 iff head(p)==head(j).
    # matmul(lhsT=ones_blk, rhs=y^2) -> per-head sum over d, already
    # broadcast across that head's 64 partitions.
    ones_blk = const.tile([128, 128], bf16)
    nc.gpsimd.memset(ones_blk, 0.0)
    nc.gpsimd.memset(ones_blk[0:HD, 0:HD], 1.0)
    nc.gpsimd.memset(ones_blk[HD:128, HD:128], 1.0)
    eps_col = const.tile([128, 1], f32)
    nc.gpsimd.memset(eps_col, EPS)

    # ln params replicated per local feature: partition p <- param[p % 64]
    g_col = const.tile([128, 1], f32)
    nc.gpsimd.dma_start(g_col, bass.AP(tensor=ln_g.tensor, offset=ln_g.offset,
                                       ap=[[0, HL], [1, HD]]))
    b_col = const.tile([128, 1], f32)
    nc.gpsimd.dma_start(b_col, bass.AP(tensor=ln_b.tensor, offset=ln_b.offset,
                                       ap=[[0, HL], [1, HD]]))
    gq_col = const.tile([128, 1], f32)
    nc.vector.tensor_scalar_mul(gq_col, g_col, SCALE)
    bq_col = const.tile([128, 1], f32)
    nc.vector.tensor_scalar_mul(bq_col, b_col, SCALE)

    # K and Q weight head-blocks arrive mean-centered from the host (LN mean
    # folded into the weights; only E[x^2] needed per tile).

    # persistent per-batch tiles (bufs=2 -> both batches in flight)
    kT = [None, None]
    qT = [None, None]
    Vt = [None, None]
    hoT = [None, None]

    # ---------------- phase generators ----------------
    def prod(b):
        """Project K|V (from x2) and Q (from x1) for batch b, with QK-LN.

        K/Q are weight-stationary (lhsT = W chunk), producing [feat, rows]
        directly in kT/qT orientation -- no PE transposes, and LDWEIGHTS
        (128 cols) hides under the N=512 matmuls.  LN per (head, token) in
        this orientation: y^2 summed over d via a block-diagonal ones
        matmul whose output is already replicated across each head's 64
        partitions, then rstd = exp(-0.5*ln(var+eps)) on ACT, folded into
        the PSUM drain together with per-partition ln_g.
        V stays x-stationary ([rows, feat] output for the AV lhsT); 4
        row-tiles pack into one PSUM bank as separate accumulation groups.
        """
        kT[b] = big_pool.tile([128, M], bf16, tag="kT", name=f"kT{b}")
        qT[b] = big_pool.tile([128, N], bf16, tag="qT", name=f"qT{b}")
        Vt[b] = big_pool.tile([128, 16, 2 * (HD + 1)], bf16, tag="V",
                              name=f"V{b}")
        # Q-rg0 first: attention chunk 0 needs qT[:, 0:512] plus the first
        # kT tiles, so this order lets the exp stream start early.
        for pidx, (src, is_q, rg) in enumerate(((x1, True, 0), (x2, False, 0),
                                                (x2, False, 1), (x1, True, 1))):
            solo = (b == 0 and pidx < 2)
            dst = qT[b] if is_q else kT[b]
            gc = gq_col if is_q else g_col
            w_sb = wq_sb if is_q else wk_sb
            # x arrives pre-transposed from the host ([B, DM, N]) so the
            # xT tiles are plain contiguous DMAs (DMA-xbar transposes
            # serialized ~7us apiece on the sync queue and gated startup).
            xT = xT_pool.tile([128, 8, 1024], bf16, tag="xT", bufs=2,
                              name=f"xT{b}{int(is_q)}{rg}")
            for fc in range(8):
                eng = nc.sync if fc % 2 == 0 else nc.gpsimd
                eng.dma_start(
                    out=xT[:, fc, :],
                    in_=src[b, fc * 128:(fc + 1) * 128,
                            rg * 1024:(rg + 1) * 1024])
            yield
            for rc2 in range(2):
                gc512 = rg * 2 + rc2          # global 512-row chunk 0..3
                rsl = slice(rc2 * 512, (rc2 + 1) * 512)
                dsl = slice(gc512 * 512, (gc512 + 1) * 512)
                yps = ps_y.tile([128, 512], f32, tag="y", bufs=2,
                                name=f"y{b}{int(is_q)}{gc512}")
                for fc in range(8):
                    nc.tensor.matmul(yps, lhsT=w_sb[:, fc, :],
                                     rhs=xT[:, fc, rsl],
                                     start=(fc == 0), stop=(fc == 7))
                ysb = nat_pool.tile([128, 512], bf16, tag="ysb", bufs=2)
                if solo:
                    nc.scalar.copy(ysb, yps)
                else:
                    nc.vector.tensor_copy(ysb, yps)
                sq = nat_pool.tile([128, 512], bf16, tag="sq", bufs=2)
                nc.vector.tensor_mul(sq, ysb, ysb)
                yield
                # per-(head, token) sum of y^2 over d, replicated across the
                # head's partitions by the block-diagonal ones matmul; var
                # tiles ride the attention "st" slots (transient, 1 bank).
                var = ps_st.tile([128, 512], f32, tag="st",
                                 name=f"var{b}{int(is_q)}{gc512}")
                nc.tensor.matmul(var, lhsT=ones_blk, rhs=sq,
                                 start=True, stop=True)
                lnv = nat_pool.tile([128, 512], f32, tag="lnv", bufs=2)
                nc.scalar.activation(lnv, var, AF.Ln, bias=eps_col,
                                     scale=1.0 / HD)
                rstd = nat_pool.tile([128, 512], bf16, tag="rstd", bufs=2)
                nc.scalar.activation(rstd, lnv, AF.Exp, scale=-0.5)
                # drain: dst = (y * g) * rstd  (ln_b == 0 fast path; the
                # general-b build adds a per-partition b_col afterwards)
                nc.vector.scalar_tensor_tensor(dst[:, dsl], yps, gc, rstd,
                                               op0=OP.mult, op1=OP.mult)
                if with_b:
                    bc = bq_col if is_q else b_col
                    nc.vector.tensor_scalar_add(dst[:, dsl], dst[:, dsl], bc)
                yield
                if not is_q:
                    vps = ps_y.tile([128, 4, 128], f32, tag="y", bufs=2,
                                    name=f"v{b}{gc512}")
                    for mi4 in range(4):
                        xsl = slice(rc2 * 512 + mi4 * 128,
                                    rc2 * 512 + (mi4 + 1) * 128)
                        for fc in range(8):
                            nc.tensor.matmul(vps[:, mi4, :],
                                             lhsT=xT[:, fc, xsl],
                                             rhs=wv_sb[:, fc, :],
                                             start=(fc == 0), stop=(fc == 7),
                                             skip_group_check=True)
                    for mi4 in range(4):
                        mt = gc512 * 4 + mi4
                        vt = Vt[b][:, mt, :]
                        nc.gpsimd.memset(vt[:, HD::HD + 1], 1.0)
                        vt3 = bass.AP(tensor=vt.tensor, offset=vt.offset,
                                      ap=[vt.ap[0], [HD + 1, HL], [1, HD]])
                        vsrc = vps[:, mi4, :].rearrange(
                            "p (h x) -> p h x", h=HL)
                        if solo and mi4 % 2 == 1:
                            nc.scalar.copy(vt3, vsrc)
                        else:
                            nc.vector.tensor_copy(vt3, vsrc)
                    yield

    def attn(b):
        """S^T -> exp -> (V|1)^T @ P^T, head-pair packed."""
        hoT[b] = big_pool.tile([128, N], bf16, tag="hoT", name=f"hoT{b}")
        for nc4 in range(4):  # 512-wide query column chunks
            ns = slice(nc4 * 512, (nc4 + 1) * 512)
            av = ps_av.tile([128, 1024], f32, tag="av", bufs=1,
                            name=f"av{b}{nc4}")
            for mc in range(16):
                mcs = slice(mc * 128, (mc + 1) * 128)
                st = ps_st.tile([128, 1024], f32, tag="st",
                                name=f"st{b}{nc4}{mc}")
                for h in range(HL):
                    nc.tensor.matmul(st[:, h * 512:(h + 1) * 512],
                                     lhsT=kT[b][h * HD:(h + 1) * HD, mcs],
                                     rhs=qT[b][h * HD:(h + 1) * HD, ns],
                                     start=True, stop=True)
                pT = pT_pool.tile([128, 1024], bf16, tag="pT")
                nc.scalar.activation(pT, st, AF.Exp)
                for h in range(HL):
                    nc.tensor.matmul(
                        av[0:HD + 1, h * 512:(h + 1) * 512],
                        lhsT=Vt[b][:, mc, h * (HD + 1):(h + 1) * (HD + 1)],
                        rhs=pT[:, h * 512:(h + 1) * 512],
                        start=(mc == 0), stop=(mc == 15),
                        skip_group_check=True)
                yield
            # drain: raw AV + normalizer -> normalized hoT chunk.
            # Both inputs of every tensor_tensor op must share a start
            # partition (BIR verifier); single-src ops (activation,
            # partition_broadcast) may cross partitions, so the Z-row work
            # lands on partition 0 and everything downstream stays 0-based.
            # The av PSUM tile is released after just the fast copy + the
            # Ln read (both ~1us), so the next chunk's AV matmuls are never
            # blocked behind the normalizer chain (keeps PE HAM-warm).
            av_sb = dr_pool.tile([128, 1024], bf16, tag="avsb")
            nc.vector.tensor_copy(av_sb[0:HD, :], av[0:HD, :])
            # 1/Z = exp(-ln(Z)) on ACT; Ln+Exp live in the same table set
            # as softmax's Exp (natural_log_exp_and_others) -> no reload.
            lnz = dr_pool.tile([128, 1024], f32, tag="lnz")
            nc.scalar.activation(lnz[0:1, :], av[HD:HD + 1, :], AF.Ln)
            yield
            rz = dr_pool.tile([128, 1024], f32, tag="rz")
            nc.scalar.activation(rz[0:1, :], lnz[0:1, :], AF.Exp, scale=-1.0)
            bcast = dr_pool.tile([128, 1024], f32, tag="bc")
            nc.gpsimd.partition_broadcast(bcast[0:HD, 0:512], rz[0:1, 0:512])
            nc.gpsimd.partition_broadcast(bcast[0:HD, 512:1024],
                                          rz[0:1, 512:1024])
            nc.vector.tensor_mul(hoT[b][0:HD, ns], av_sb[0:HD, 0:512],
                                 bcast[0:HD, 0:512])
            nc.gpsimd.tensor_mul(hoT[b][HD:128, ns], av_sb[0:HD, 512:1024],
                                 bcast[0:HD, 512:1024])
            if DEBUG_DUMPS and b == 0 and nc4 in (0, 1):
                nc.sync.dma_start(aps[f"dbg{nc4}_avsb"], av_sb)
                nc.sync.dma_start(aps[f"dbg{nc4}_rz"], rz)
                nc.sync.dma_start(aps[f"dbg{nc4}_bc"], bcast)
                nc.sync.dma_start(aps[f"dbg{nc4}_ho"], hoT[b][:, ns])
            if DEBUG_DUMPS and b == 0 and nc4 == 3:
                nc.sync.dma_start(aps["dbg_kT"], kT[0])
                nc.sync.dma_start(aps["dbg_qT"], qT[0])
                nc.sync.dma_start(
                    aps["dbg_Vt"], Vt[0].rearrange("p a b -> p (a b)"))
            yield

    def outp_unit(b, nt, oc):
        """One output-projection tile: matmul + fp16 drain + store.

        fps reuses the prod-phase "y" PSUM slots (same 1-bank size, and the
        last y-tag use — prod(1) — is fully emitted before the first outp
        unit), keeping total PSUM at 8 banks."""
        fps = ps_y.tile([128, 512], f32, tag="y", bufs=2,
                        name=f"fps{b}{nt}{oc}")
        nc.tensor.matmul(fps,
                         lhsT=hoT[b][:, nt * 128:(nt + 1) * 128],
                         rhs=wp_sb[:, oc * 512:(oc + 1) * 512],
                         start=True, stop=True)
        osb = out_pool.tile([128, 512], f16, tag="osb")
        nc.vector.tensor_copy(osb, fps)
        nc.sync.dma_start(
            out[b, nt * 128:(nt + 1) * 128, oc * 512:(oc + 1) * 512],
            osb)

    def run_all(g):
        for _ in g:
            pass

    def run_n(g, n):
        for _ in range(n):
            if next(g, _SENTINEL) is _SENTINEL:
                return False
        return True

    def interleave(ga, gb, ka, kb):
        """Alternate ka steps of ga with kb steps of gb until both drain."""
        alive_a, alive_b = True, True
        while alive_a or alive_b:
            for _ in range(ka):
                if alive_a:
                    alive_a = next(ga, _SENTINEL) is not _SENTINEL
            for _ in range(kb):
                if alive_b:
                    alive_b = next(gb, _SENTINEL) is not _SENTINEL

    def attn1_with_outp():
        """attn(1) with outp(0) and outp(1) interleaved chunk-by-chunk.

        outp(0)'s hoT is fully written before this phase; outp(1) chunks are
        appended to the work queue as attn(1) finishes each 512-column chunk
        (emission order matches the dependency order, so Tile's tracking
        stays correct)."""
        units = [(0, nt, oc) for nt in range(16) for oc in range(2)]
        ga = attn(1)
        step = 0
        nc4_done = 0
        alive = True
        while alive or units:
            if alive:
                alive = next(ga, _SENTINEL) is not _SENTINEL
                step += 1
                # 18 yields per nc4 chunk (16 mc + 2 drain)
                if alive and step % 18 == 0:
                    for nt in range(nc4_done * 4, nc4_done * 4 + 4):
                        for oc in range(2):
                            units.append((1, nt, oc))
                    nc4_done += 1
                if not alive:
                    while nc4_done < 4:
                        for nt in range(nc4_done * 4, nc4_done * 4 + 4):
                            for oc in range(2):
                                units.append((1, nt, oc))
                        nc4_done += 1
            if units:
                outp_unit(*units.pop(0))

    # Emission IS the dependency order (Tile tracks emission-ordered deps)
    # and largely the execution order, so attn(0) must be emitted early to
    # start the exp stream early.  prod(0) passes 1-2 (Q-rg0: 5 yields,
    # KV-rg0: 7 yields) give attn chunk 0 its inputs for mc 0-7; chunk 0's
    # mc>=8 readers need pass 3 (KV-rg1, 7 yields), so attn paces 1:1
    # behind the rest of prod(0) (12 yields) before going attn-only.
    gp0 = prod(0)
    run_n(gp0, 12)              # Q-rg0 + KV-rg0 complete
    ga0 = attn(0)
    for _ in range(12):
        run_n(ga0, 1)
        run_n(gp0, 1)
    run_all(gp0)                # safety drain (no-op when counts match)
    interleave(ga0, prod(1), 2, 1)
    attn1_with_outp()


def _build(with_b=False):
    global _COMPILED
    if _COMPILED is not None and _COMPILED[0] == with_b:
        return _COMPILED[1]
    import concourse.tile as tile
    from concourse import bacc, mybir
    from concourse.hw_specs import get_activation_tables

    # Pin Exp/Ln/Copy/Identity/Square to the one table set that has them all
    # (natural_log_exp_and_others); otherwise the table-load inserter
    # ping-pongs between exp_and_others and the ln set (1.3us per reload,
    # on the softmax critical path).  Set ids are positional, so entries are
    # edited in place, never removed.
    _AF = mybir.ActivationFunctionType
    _tabs = get_activation_tables("gen3")
    for _name, _fns in _tabs.items():
        if _name != "natural_log_exp_and_others":
            for _f in (_AF.Exp, _AF.Ln, _AF.Copy, _AF.Identity, _AF.Square):
                _fns.discard(_f)

    nc = bacc.Bacc("TRN2", target_bir_lowering=False, debug=False,
                   enable_asserts=False)
    bf16 = mybir.dt.bfloat16
    f32 = mybir.dt.float32
    f16 = mybir.dt.float16
    aps = {
        "x1": nc.dram_tensor("x1", [B, DM, N], bf16, kind="ExternalInput").ap(),
        "x2": nc.dram_tensor("x2", [B, DM, M], bf16, kind="ExternalInput").ap(),
        "wqT": nc.dram_tensor("wqT", [DM, LOC], bf16, kind="ExternalInput").ap(),
        "wkT": nc.dram_tensor("wkT", [DM, LOC], bf16, kind="ExternalInput").ap(),
        "wvT": nc.dram_tensor("wvT", [DM, LOC], bf16, kind="ExternalInput").ap(),
        "wp": nc.dram_tensor("wp", [LOC, DM], bf16, kind="ExternalInput").ap(),
        "ln_g": nc.dram_tensor("ln_g", [HD], f32, kind="ExternalInput").ap(),
        "ln_b": nc.dram_tensor("ln_b", [HD], f32, kind="ExternalInput").ap(),
        "out": nc.dram_tensor("out", [B, N, DM], f16, kind="ExternalOutput").ap(),
    }
    if DEBUG_DUMPS:
        for c in (0, 1):
            aps[f"dbg{c}_avsb"] = nc.dram_tensor(
                f"dbg{c}_avsb", [128, 1024], bf16, kind="ExternalOutput").ap()
            aps[f"dbg{c}_rz"] = nc.dram_tensor(
                f"dbg{c}_rz", [128, 1024], f32, kind="ExternalOutput").ap()
            aps[f"dbg{c}_bc"] = nc.dram_tensor(
                f"dbg{c}_bc", [128, 1024], f32, kind="ExternalOutput").ap()
            aps[f"dbg{c}_ho"] = nc.dram_tensor(
                f"dbg{c}_ho", [128, 512], bf16, kind="ExternalOutput").ap()
        aps["dbg_kT"] = nc.dram_tensor(
            "dbg_kT", [128, M], bf16, kind="ExternalOutput").ap()
        aps["dbg_qT"] = nc.dram_tensor(
            "dbg_qT", [128, N], bf16, kind="ExternalOutput").ap()
        aps["dbg_Vt"] = nc.dram_tensor(
            "dbg_Vt", [128, 16 * 130], bf16, kind="ExternalOutput").ap()
    with tile.TileContext(nc) as tc, ExitStack() as ctx:
        _emit(ctx, tc, aps, with_b)
    nc.compile()
    _COMPILED = (with_b, nc)
    return nc


def kernel(x1, x2, Wq, Wk, Wv, Wp, bp, ln_g, ln_b):
    global LAST_RESULT
    from concourse.bass_utils import run_bass_kernel_spmd

    nc = _build(with_b=bool(np.any(np.asarray(ln_b, dtype=np.float32))))
    bf = ml_dtypes.bfloat16
    # Host-side transpose to [B, DM, N]: device consumes x only in
    # transposed form, and plain DMAs are ~6x faster than DMA-xbar
    # transposes on the sync queue.
    x1b = np.ascontiguousarray(
        np.asarray(x1, dtype=np.float32).transpose(0, 2, 1)).astype(bf)
    x2b = np.ascontiguousarray(
        np.asarray(x2, dtype=np.float32).transpose(0, 2, 1)).astype(bf)
    Wq = np.asarray(Wq, dtype=np.float32)
    Wk = np.asarray(Wk, dtype=np.float32)
    Wv = np.asarray(Wv, dtype=np.float32)
    Wp = np.asarray(Wp, dtype=np.float32)
    # Fold the LN mean into the K/Q weights: subtract each head's mean over
    # its 64 output features (torch Linear rows), so projections come out
    # zero-mean per head and the device only needs E[x^2].
    Wq = (Wq.reshape(H, HD, DM) -
          Wq.reshape(H, HD, DM).mean(axis=1, keepdims=True)).reshape(DM, DM)
    Wk = (Wk.reshape(H, HD, DM) -
          Wk.reshape(H, HD, DM).mean(axis=1, keepdims=True)).reshape(DM, DM)
    ln_g32 = np.ascontiguousarray(np.asarray(ln_g, dtype=np.float32))
    ln_b32 = np.ascontiguousarray(np.asarray(ln_b, dtype=np.float32))

    in_maps = []
    for c in range(NCORES):
        hs = slice(c * LOC, (c + 1) * LOC)
        in_maps.append({
            "x1": x1b,
            "x2": x2b,
            "wqT": np.ascontiguousarray(Wq[hs, :].T).astype(bf),
            "wkT": np.ascontiguousarray(Wk[hs, :].T).astype(bf),
            "wvT": np.ascontiguousarray(Wv[hs, :].T).astype(bf),
            "wp": np.ascontiguousarray(Wp[:, hs].T).astype(bf),
            "ln_g": ln_g32,
            "ln_b": ln_b32,
        })

    res = run_bass_kernel_spmd(nc, in_maps, core_ids=list(range(NCORES)))
    LAST_RESULT = res
    acc = np.zeros((B, N, DM), dtype=np.float32)
    for r in res.results:
        acc += np.asarray(r["out"], dtype=np.float32)
    acc += np.asarray(bp, dtype=np.float32)
    return acc

